# revision 14
# baseline (speedup 1.0000x reference)
"""Trainium2 Bass kernel for nn_EdgeDecoder (GNN edge decoder, 2 relations).

Strategy (data-parallel over edges, 8 NeuronCores):
  - Shard the 500k edges of each relation across 8 cores (62500/core).
  - Per (core, relation, sub-shard): host remaps node indices into a compact
    per-shard embedding table (np.unique) so indices fit int16, which enables
    the SWDGE dma_gather instruction (one descriptor per edge instead of one
    instruction per 128 edges). Tables are cast to fp16 on host.
  - On device, per 4096-edge chunk: dma_gather user/item rows (fp16, 256B
    rows), PE-transpose 128x128 blocks to get [dim, edge] layout, then
      hT = relu(W1u^T huT + W1v^T hvT + b1)   (fp16 matmuls, f32 PSUM)
      logits = W2^T hT + b2                   (fp16 matmul,  f32 PSUM)
    and DMA the f32 logits back per chunk. Logits stay sharded; host
    reassembles the full [500000] outputs.
"""
import sys

if "/opt/trn_rl_repo" not in sys.path:
    sys.path.insert(0, "/opt/trn_rl_repo")

import numpy as np

P = 128
D = 128
HID = 256
E = 500000
NCORES = 8
EPC = E // NCORES          # 62500 edges per core per relation
GCH = 4096                 # edges per gather chunk
CCH = 512                  # edges per compute chunk
NREL = 2

_PROGRAM_CACHE = {}
LAST_RESULTS = None


def _build_program(nsub, nchunk, tabrows, subl):
    import concourse.bacc as bacc
    import concourse.mybir as mybir
    from concourse.tile import TileContext

    f16, f32, i16 = mybir.dt.float16, mybir.dt.float32, mybir.dt.int16
    subpad = nchunk * GCH
    # real (non-pad) index count per chunk; pads are -1 and the SWDGE ucode
    # stops descriptor generation at the last non-negative index
    counts = [min(GCH, subl - c * GCH) for c in range(nchunk)]

    nc = bacc.Bacc("TRN2", target_bir_lowering=False, debug=False,
                   num_swdge_queues=4)

    tabs, idxs_d, outs = {}, {}, {}
    for r in range(NREL):
        for s in range(nsub):
            tabs[("u", r, s)] = nc.dram_tensor(
                f"ut{r}_{s}", [tabrows, D], f16, kind="ExternalInput")
            tabs[("v", r, s)] = nc.dram_tensor(
                f"vt{r}_{s}", [tabrows, D], f16, kind="ExternalInput")
            idxs_d[("u", r, s)] = nc.dram_tensor(
                f"ui{r}_{s}", [nchunk, P, GCH // 16], i16, kind="ExternalInput")
            idxs_d[("v", r, s)] = nc.dram_tensor(
                f"vi{r}_{s}", [nchunk, P, GCH // 16], i16, kind="ExternalInput")
        outs[r] = nc.dram_tensor(f"o{r}", [nsub, subpad], f32,
                                 kind="ExternalOutput")
    w1u_d = [nc.dram_tensor(f"w1u{r}", [D, HID], f16, kind="ExternalInput")
             for r in range(NREL)]
    w1v_d = [nc.dram_tensor(f"w1v{r}", [D, HID], f16, kind="ExternalInput")
             for r in range(NREL)]
    w2_d = [nc.dram_tensor(f"w2{r}", [P, 2], f16, kind="ExternalInput")
            for r in range(NREL)]
    b1_d = [nc.dram_tensor(f"b1{r}", [P, 2], f32, kind="ExternalInput")
            for r in range(NREL)]
    b2_d = [nc.dram_tensor(f"b2{r}", [1, 1], f32, kind="ExternalInput")
            for r in range(NREL)]
    id_d = nc.dram_tensor("ident", [P, P], f16, kind="ExternalInput")

    with TileContext(nc) as tc:
        with tc.tile_pool(name="sbw", bufs=1) as sbw, \
             tc.tile_pool(name="sbi", bufs=6) as sbi, \
             tc.tile_pool(name="sbg", bufs=4) as sbg, \
             tc.tile_pool(name="sbt", bufs=3) as sbt, \
             tc.tile_pool(name="sbh", bufs=4) as sbh, \
             tc.tile_pool(name="sblog", bufs=2) as sblog, \
             tc.tile_pool(name="pt", bufs=2, space="PSUM") as pt, \
             tc.tile_pool(name="ph", bufs=3, space="PSUM") as ph, \
             tc.tile_pool(name="pl", bufs=2, space="PSUM") as pl:

            w1u_t, w1v_t, w2_t, b1_t, b2_t = [], [], [], [], []
            for r in range(NREL):
                t = sbw.tile([D, HID], f16, tag=f"w1u{r}")
                nc.sync.dma_start(out=t[:], in_=w1u_d[r].ap()[:])
                w1u_t.append(t)
                t = sbw.tile([D, HID], f16, tag=f"w1v{r}")
                nc.sync.dma_start(out=t[:], in_=w1v_d[r].ap()[:])
                w1v_t.append(t)
                t = sbw.tile([P, 2], f16, tag=f"w2{r}")
                nc.sync.dma_start(out=t[:], in_=w2_d[r].ap()[:])
                w2_t.append(t)
                t = sbw.tile([P, 2], f32, tag=f"b1{r}")
                nc.sync.dma_start(out=t[:], in_=b1_d[r].ap()[:])
                b1_t.append(t)
                t = sbw.tile([1, 1], f32, tag=f"b2{r}")
                nc.sync.dma_start(out=t[:], in_=b2_d[r].ap()[:])
                b2_t.append(t)
            ident = sbw.tile([P, P], f16, tag="ident")
            nc.sync.dma_start(out=ident[:], in_=id_d.ap()[:])

            q = 0
            from contextlib import nullcontext
            for r in range(NREL):
                for s in range(nsub):
                    for c in range(nchunk):
                        # front-load the very first gathers so the SWDGE
                        # cores start before the weight loads finish
                        prio = tc.high_priority() if (r, s, c) == (0, 0, 0) \
                            else nullcontext()
                        with prio:
                            ui_t = sbi.tile([P, GCH // 16], i16, tag="ui")
                            nc.sync.dma_start(out=ui_t[:],
                                              in_=idxs_d[("u", r, s)].ap()[c])
                            vi_t = sbi.tile([P, GCH // 16], i16, tag="vi")
                            nc.sync.dma_start(out=vi_t[:],
                                              in_=idxs_d[("v", r, s)].ap()[c])
                            gu = sbg.tile([P, GCH // P, D], f16, tag="gu")
                            nc.gpsimd.dma_gather(
                                gu[:], tabs[("u", r, s)].ap()[:], ui_t[:],
                                GCH, counts[c], D, single_packet=False,
                                queue_num=q % 4)
                            gv = sbg.tile([P, GCH // P, D], f16, tag="gv")
                            nc.gpsimd.dma_gather(
                                gv[:], tabs[("v", r, s)].ap()[:], vi_t[:],
                                GCH, counts[c], D, single_packet=False,
                                queue_num=(q + 1) % 4)
                        q += 2

                        log_sb = sblog.tile([1, GCH], f32, tag="log")
                        ncc = -(-counts[c] // CCH)
                        for cc in range(ncc):
                            ptu = pt.tile([P, CCH], f16, tag="pt")
                            for j in range(CCH // P):
                                nc.tensor.transpose(
                                    out=ptu[:, j * P:(j + 1) * P],
                                    in_=gu[:, cc * (CCH // P) + j, :],
                                    identity=ident[:])
                            tu = sbt.tile([P, CCH], f16, tag="tu")
                            nc.vector.tensor_copy(out=tu[:], in_=ptu[:])
                            ptv = pt.tile([P, CCH], f16, tag="pt")
                            for j in range(CCH // P):
                                nc.tensor.transpose(
                                    out=ptv[:, j * P:(j + 1) * P],
                                    in_=gv[:, cc * (CCH // P) + j, :],
                                    identity=ident[:])
                            tv = sbt.tile([P, CCH], f16, tag="tv")
                            nc.vector.tensor_copy(out=tv[:], in_=ptv[:])

                            hts = []
                            for hc in range(2):
                                php = ph.tile([P, CCH], f32, tag="ph")
                                nc.tensor.matmul(
                                    out=php[:],
                                    lhsT=w1u_t[r][:, hc * P:(hc + 1) * P],
                                    rhs=tu[:], start=True, stop=False)
                                nc.tensor.matmul(
                                    out=php[:],
                                    lhsT=w1v_t[r][:, hc * P:(hc + 1) * P],
                                    rhs=tv[:], start=False, stop=True)
                                ht = sbh.tile([P, CCH], f16, tag="ht")
                                nc.scalar.activation(
                                    out=ht[:], in_=php[:],
                                    func=mybir.ActivationFunctionType.Relu,
                                    bias=b1_t[r][:, hc:hc + 1])
                                hts.append(ht)
                            plt = pl.tile([1, CCH], f32, tag="pl")
                            nc.tensor.matmul(out=plt[:], lhsT=w2_t[r][:, 0:1],
                                             rhs=hts[0][:], start=True, stop=False)
                            nc.tensor.matmul(out=plt[:], lhsT=w2_t[r][:, 1:2],
                                             rhs=hts[1][:], start=False, stop=True)
                            nc.scalar.activation(
                                out=log_sb[:, cc * CCH:(cc + 1) * CCH],
                                in_=plt[:],
                                func=mybir.ActivationFunctionType.Identity,
                                bias=b2_t[r][:])
                        nc.sync.dma_start(
                            out=outs[r].ap()[s:s + 1,
                                             c * GCH:c * GCH + ncc * CCH],
                            in_=log_sb[:, :ncc * CCH])
    nc.compile()
    return nc


def _wrap16(idx16, nchunk):
    """[subpad] int16 -> [nchunk, 128, GCH//16]: stream pos g of chunk c sits
    at partition g%16 (replicated to all 8 Q7 core groups), column g//16."""
    a = idx16.reshape(nchunk, GCH // 16, 16)
    a = np.swapaxes(a, 1, 2)                       # [nchunk, 16, GCH//16]
    return np.tile(a, (1, 8, 1)).copy()            # [nchunk, 128, GCH//16]


def _prep(user_embed, item_embed, u_clicks, v_clicks, u_buys, v_buys,
          W1_clicks, b1_clicks, W2_clicks, b2_clicks,
          W1_buys, b1_buys, W2_buys, b2_buys):
    user_embed = np.asarray(user_embed, dtype=np.float32)
    item_embed = np.asarray(item_embed, dtype=np.float32)
    rels = [
        (np.asarray(u_clicks), np.asarray(v_clicks),
         np.asarray(W1_clicks, np.float32), np.asarray(b1_clicks, np.float32),
         np.asarray(W2_clicks, np.float32), np.asarray(b2_clicks, np.float32)),
        (np.asarray(u_buys), np.asarray(v_buys),
         np.asarray(W1_buys, np.float32), np.asarray(b1_buys, np.float32),
         np.asarray(W2_buys, np.float32), np.asarray(b2_buys, np.float32)),
    ]
    user16 = user_embed.astype(np.float16)
    item16 = item_embed.astype(np.float16)

    # pick nsub so every sub-shard's unique index count fits int16
    nsub = 2
    while True:
        subl = EPC // nsub
        ok = True
        for r in range(NREL):
            u_all, v_all = rels[r][0], rels[r][1]
            for k in range(NCORES):
                for s in range(nsub):
                    lo = k * EPC + s * subl
                    hi = lo + subl
                    if len(np.unique(u_all[lo:hi])) > 32700 or \
                       len(np.unique(v_all[lo:hi])) > 32700:
                        ok = False
                        break
                if not ok:
                    break
            if not ok:
                break
        if ok:
            break
        nsub *= 2
        if nsub > 16:
            raise RuntimeError("index space too dense for int16 gather")
    subl = EPC // nsub
    nchunk = -(-subl // GCH)          # chunks per sub-shard
    subpad = nchunk * GCH
    tabrows = 32768

    in_maps = []
    for k in range(NCORES):
        m = {"ident": np.eye(P, dtype=np.float16)}
        for r in range(NREL):
            u_all, v_all, W1, b1, W2, b2 = rels[r]
            m[f"w1u{r}"] = W1[:D].astype(np.float16)
            m[f"w1v{r}"] = W1[D:].astype(np.float16)
            m[f"w2{r}"] = W2.reshape(2, P).T.astype(np.float16).copy()
            m[f"b1{r}"] = b1.reshape(2, P).T.astype(np.float32).copy()
            m[f"b2{r}"] = b2.reshape(1, 1).astype(np.float32)
            for s in range(nsub):
                lo = k * EPC + s * subl
                hi = lo + subl
                for tag, idx_all, tab16 in (("u", u_all, user16),
                                            ("v", v_all, item16)):
                    idx = np.asarray(idx_all[lo:hi], dtype=np.int64)
                    uniq, inv = np.unique(idx, return_inverse=True)
                    comp = np.zeros((tabrows, D), np.float16)
                    comp[:len(uniq)] = tab16[uniq]
                    inv16 = np.full(subpad, -1, np.int16)
                    inv16[:subl] = inv.astype(np.int16)
                    m[f"{tag}t{r}_{s}"] = comp
                    m[f"{tag}i{r}_{s}"] = _wrap16(inv16, nchunk)
        in_maps.append(m)
    return nsub, nchunk, subl, subpad, tabrows, in_maps


def make_in_maps(np_inputs):
    """For external harnesses: per-core input maps for the cached program."""
    _, _, _, _, _, in_maps = _prep(**np_inputs)
    return in_maps


def kernel(**inputs):
    global LAST_RESULTS
    from concourse import bass_utils

    nsub, nchunk, subl, subpad, tabrows, in_maps = _prep(**inputs)

    key = (nsub, nchunk, tabrows, subl)
    if key not in _PROGRAM_CACHE:
        _PROGRAM_CACHE[key] = _build_program(nsub, nchunk, tabrows, subl)
    nc = _PROGRAM_CACHE[key]

    res = bass_utils.run_bass_kernel_spmd(nc, in_maps, core_ids=list(range(NCORES)))
    LAST_RESULTS = res

    outs = []
    for r in range(NREL):
        full = np.empty(E, np.float32)
        for k in range(NCORES):
            o = res.results[k][f"o{r}"]          # [nsub, subpad]
            for s in range(nsub):
                lo = k * EPC + s * subl
                full[lo:lo + subl] = o[s, :subl]
        outs.append(full)
    return outs[0], outs[1]
